# revision 2
# baseline (speedup 1.0000x reference)
"""ExpertRouter (MoE routing) Trainium2 kernel.

Data-parallel over tokens across 8 NeuronCores. Each core computes, for its
2048-token shard: router logits x@w+b ([2048, 64] fp32), top-2 expert
indices, and softmax gates over the top-2 logits.

Host-side prep per shard: x is transposed to [d_model, tokens] so DMA loads
land [128 d_model, tokens] tiles directly (d_model on SBUF partitions = the
matmul contraction dim), avoiding any on-chip transpose. The tiny router
weight is replicated to all cores, packed [128, KC*64].

Per 128-token tile the PE accumulates 32 fp32 matmuls (K=128 each) into a
[128 tokens, 64 experts] PSUM tile; DVE adds bias, the DVE sort hardware
(max / max_index) yields descending top-8 values+indices, and gates come
from g0 = 1/(1+e^(l1-l0)), g1 = e^(l1-l0)*g0.
"""

import numpy as np

D_MODEL = 4096
NUM_EXPERTS = 64
TOP_K = 2
N_CORES = 8
P = 128  # partitions / matmul K-chunk / token-tile size


def build_program(d_model=D_MODEL, t_core=2048, n_experts=NUM_EXPERTS,
                  tiles_per_pass=8, x_bufs=4):
    """Build the per-core Bass/Tile program. Returns the compiled Bacc module."""
    import concourse.bacc as bacc
    import concourse.tile as tile
    import concourse.mybir as mybir

    f32 = mybir.dt.float32
    u32 = mybir.dt.uint32

    kc_n = d_model // P          # contraction chunks
    nt = t_core // P             # token tiles per core
    tiles_per_pass = min(tiles_per_pass, nt)
    assert nt % tiles_per_pass == 0
    n_pass = nt // tiles_per_pass

    nc = bacc.Bacc("TRN2", target_bir_lowering=False, debug=False,
                   num_devices=N_CORES)

    xt_d = nc.dram_tensor("xt", [d_model, t_core], f32, kind="ExternalInput").ap()
    wp_d = nc.dram_tensor("wp", [P, kc_n * n_experts], f32, kind="ExternalInput").ap()
    b_d = nc.dram_tensor("b128", [P, n_experts], f32, kind="ExternalInput").ap()
    logits_d = nc.dram_tensor("logits", [t_core, n_experts], f32, kind="ExternalOutput").ap()
    idx_d = nc.dram_tensor("idx", [t_core, TOP_K], u32, kind="ExternalOutput").ap()
    gates_d = nc.dram_tensor("gates", [t_core, TOP_K], f32, kind="ExternalOutput").ap()

    with tile.TileContext(nc) as tc:
        with (
            tc.tile_pool(name="consts", bufs=1) as consts,
            tc.tile_pool(name="xp", bufs=x_bufs) as xp,
            tc.tile_pool(name="ps", bufs=tiles_per_pass, space="PSUM") as psp,
            tc.tile_pool(name="small", bufs=3) as small,
        ):
            w_sb = consts.tile([P, kc_n * n_experts], f32)
            nc.sync.dma_start(out=w_sb[:], in_=wp_d[:])
            b_sb = consts.tile([P, n_experts], f32)
            nc.sync.dma_start(out=b_sb[:], in_=b_d[:])

            logits_acc = consts.tile([P, nt * n_experts], f32)
            idx_acc = consts.tile([P, nt * TOP_K], u32)
            gates_acc = consts.tile([P, nt * TOP_K], f32)

            for ph in range(n_pass):
                tw = tiles_per_pass * P  # tokens in this pass
                ps_tiles = [psp.tile([P, n_experts], f32, tag="ps",
                                     name=f"ps_{ph}_{t}")
                            for t in range(tiles_per_pass)]
                for kc in range(kc_n):
                    xt_t = xp.tile([P, tw], f32, tag="xt")
                    nc.sync.dma_start(
                        out=xt_t[:],
                        in_=xt_d[kc * P:(kc + 1) * P, ph * tw:(ph + 1) * tw])
                    for t in range(tiles_per_pass):
                        nc.tensor.matmul(
                            ps_tiles[t][:],
                            lhsT=xt_t[:, t * P:(t + 1) * P],
                            rhs=w_sb[:, kc * n_experts:(kc + 1) * n_experts],
                            start=(kc == 0), stop=(kc == kc_n - 1))

                for t in range(tiles_per_pass):
                    g = ph * tiles_per_pass + t
                    lg = logits_acc[:, g * n_experts:(g + 1) * n_experts]
                    nc.vector.tensor_add(lg, ps_tiles[t][:], b_sb[:])

                    maxv = small.tile([P, 8], f32, tag="maxv")
                    nc.vector.max(out=maxv[:], in_=lg)
                    idx8 = small.tile([P, 8], u32, tag="idx8")
                    nc.vector.max_index(out=idx8[:], in_max=maxv[:], in_values=lg)
                    nc.vector.tensor_copy(idx_acc[:, g * 2:g * 2 + 2], idx8[:, 0:2])

                    # top-2 softmax: d = l1 - l0 (<= 0), e = exp(d)
                    # g0 = 1/(1+e), g1 = e/(1+e)
                    ed = small.tile([P, 1], f32, tag="ed")
                    nc.vector.tensor_sub(ed[:], maxv[:, 1:2], maxv[:, 0:1])
                    nc.scalar.activation(ed[:], ed[:], mybir.ActivationFunctionType.Exp)
                    s = small.tile([P, 1], f32, tag="s")
                    nc.vector.tensor_scalar_add(s[:], ed[:], 1.0)
                    g0 = gates_acc[:, g * 2:g * 2 + 1]
                    nc.vector.reciprocal(g0, s[:])
                    nc.vector.tensor_mul(gates_acc[:, g * 2 + 1:g * 2 + 2], ed[:], g0)

            # Batched stores: DRAM [(i p), e] <- SBUF [p, (i e)]
            nc.sync.dma_start(
                out=logits_d.rearrange("(i p) e -> p i e", p=P),
                in_=logits_acc[:].rearrange("p (i e) -> p i e", e=n_experts))
            nc.sync.dma_start(
                out=idx_d.rearrange("(i p) e -> p i e", p=P),
                in_=idx_acc[:].rearrange("p (i e) -> p i e", e=TOP_K))
            nc.sync.dma_start(
                out=gates_d.rearrange("(i p) e -> p i e", p=P),
                in_=gates_acc[:].rearrange("p (i e) -> p i e", e=TOP_K))

    nc.compile()
    return nc


def make_in_maps(x, w, b, d_model, t_core, n_experts):
    """Host-side shard + layout prep. x: [tokens_total, d_model]."""
    kc_n = d_model // P
    wp = np.ascontiguousarray(
        w.reshape(kc_n, P, n_experts).transpose(1, 0, 2).reshape(P, kc_n * n_experts))
    b128 = np.ascontiguousarray(np.broadcast_to(b, (P, n_experts)), dtype=np.float32)
    in_maps = []
    for i in range(N_CORES):
        shard = x[i * t_core:(i + 1) * t_core]
        xt = np.ascontiguousarray(shard.T)
        in_maps.append({"xt": xt, "wp": wp, "b128": b128})
    return in_maps


_CACHE = {}


def _run(x, w, b, d_model, t_core, n_experts, trace=False):
    from concourse.bass_utils import run_bass_kernel_spmd
    key = (d_model, t_core, n_experts)
    if key not in _CACHE:
        _CACHE[key] = build_program(d_model, t_core, n_experts)
    nc = _CACHE[key]
    in_maps = make_in_maps(x, w, b, d_model, t_core, n_experts)
    res = run_bass_kernel_spmd(nc, in_maps, core_ids=list(range(N_CORES)),
                               trace=trace)
    logits = np.concatenate([res.results[i]["logits"] for i in range(N_CORES)])
    idx = np.concatenate([res.results[i]["idx"] for i in range(N_CORES)])
    gates = np.concatenate([res.results[i]["gates"] for i in range(N_CORES)])
    return logits, idx.astype(np.int32), gates, res


def kernel(x, w, b):
    x = np.asarray(x, dtype=np.float32)
    w = np.asarray(w, dtype=np.float32)
    b = np.asarray(b, dtype=np.float32)
    bsz, seq, d_model = x.shape
    n_experts = w.shape[1]
    tokens = bsz * seq
    t_core = tokens // N_CORES
    logits, idx, gates, _ = _run(
        x.reshape(tokens, d_model), w, b, d_model, t_core, n_experts)
    return (logits.reshape(bsz, seq, n_experts),
            idx.reshape(bsz, seq, TOP_K),
            gates.reshape(bsz, seq, TOP_K))


# revision 12
# speedup vs baseline: 1.8270x; 1.8270x over previous
"""ExpertRouter (MoE routing) Trainium2 kernel.

Data-parallel over tokens across 8 NeuronCores. Each core computes, for its
2048-token shard: router logits x@w+b ([2048, 64] fp32), top-2 expert
indices, and softmax gates over the top-2 logits.

Host-side prep per shard: x is transposed to [d_model, tokens] so DMA loads
land [128 d_model, tokens] tiles directly (d_model on SBUF partitions = the
matmul contraction dim), avoiding any on-chip transpose. The tiny router
weight is replicated to all cores, packed [128, KC*64].

Per 128-token tile the PE accumulates 32 fp32 matmuls (K=128 each) into a
[128 tokens, 64 experts] PSUM tile; DVE adds bias, the DVE sort hardware
(max / max_index) yields descending top-8 values+indices, and gates come
from g0 = 1/(1+e^(l1-l0)), g1 = e^(l1-l0)*g0.
"""

import numpy as np

D_MODEL = 4096
NUM_EXPERTS = 64
TOP_K = 2
N_CORES = 8
P = 128  # partitions / matmul K-chunk / token-tile size


def build_program_b(d_model=D_MODEL, t_core=2048, n_experts=NUM_EXPERTS,
                    tokens_per_pass=1024, cg_w=512, x_bufs=8, use_f32r=False,
                    warmup_mms=20, permuted_out=True):
    """Design B: w stationary [128, 64] (reused), x moving at float32r full
    rate. PSUM gets logits^T [64, cg_w]; a small PE transpose flips each
    128-token block back to [128 tokens, 64 experts] for the top-k path."""
    import concourse.bacc as bacc
    import concourse.tile as tile
    import concourse.mybir as mybir

    f32 = mybir.dt.float32
    fmm = mybir.dt.float32r if use_f32r else f32
    u32 = mybir.dt.uint32

    kc_n = d_model // P
    nt = t_core // P
    assert t_core % tokens_per_pass == 0 and tokens_per_pass % cg_w == 0
    n_pass = t_core // tokens_per_pass
    n_cg = tokens_per_pass // cg_w
    tpc = cg_w // P  # token tiles per column group

    nc = bacc.Bacc("TRN2", target_bir_lowering=False, debug=False,
                   num_devices=N_CORES)

    bf16 = mybir.dt.bfloat16
    xt_d = nc.dram_tensor("xt", [d_model, t_core], fmm, kind="ExternalInput").ap()
    wp_d = nc.dram_tensor("wp", [P, kc_n * n_experts], fmm, kind="ExternalInput").ap()
    b_d = nc.dram_tensor("b128", [P, n_experts], f32, kind="ExternalInput").ap()
    id_d = nc.dram_tensor("ident", [n_experts, n_experts], f32, kind="ExternalInput").ap()
    if permuted_out:
        logits_d = nc.dram_tensor("logits", [P, nt * n_experts], f32, kind="ExternalOutput").ap()
        idx_d = nc.dram_tensor("idx", [P, nt * TOP_K], u32, kind="ExternalOutput").ap()
        gates_d = nc.dram_tensor("gates", [P, nt * TOP_K], f32, kind="ExternalOutput").ap()
    else:
        logits_d = nc.dram_tensor("logits", [t_core, n_experts], f32, kind="ExternalOutput").ap()
        idx_d = nc.dram_tensor("idx", [t_core, TOP_K], u32, kind="ExternalOutput").ap()
        gates_d = nc.dram_tensor("gates", [t_core, TOP_K], f32, kind="ExternalOutput").ap()

    with tile.TileContext(nc) as tc:
        with (
            tc.tile_pool(name="consts", bufs=1) as consts,
            tc.tile_pool(name="xp", bufs=x_bufs) as xp,
            tc.tile_pool(name="psB", bufs=2 * n_cg, space="PSUM") as psB,
            tc.tile_pool(name="psT", bufs=2, space="PSUM") as psT,
            tc.tile_pool(name="lgp", bufs=2) as lgp,
            tc.tile_pool(name="small", bufs=3) as small,
        ):
            if warmup_mms:
                # HAM pre-warm: ~20 bf16 MMs (~4us busy) ahead of the first
                # real matmul so fp32 streams run at 2.4 GHz from the start.
                wt = consts.tile([P, 512], bf16)
                nc.vector.memset(wt[:], 0.0)
                wps = psT.tile([P, 512], f32, tag="warm")
                for i in range(warmup_mms):
                    nc.tensor.matmul(wps[:], lhsT=wt[:, 0:P], rhs=wt[:],
                                     start=True, stop=True)
            w_sb = consts.tile([P, kc_n * n_experts], fmm)
            nc.sync.dma_start(out=w_sb[:], in_=wp_d[:])
            b_sb = consts.tile([P, n_experts], f32)
            nc.sync.dma_start(out=b_sb[:], in_=b_d[:])
            id_sb = consts.tile([n_experts, n_experts], f32)
            nc.sync.dma_start(out=id_sb[:], in_=id_d[:])

            logits_acc = consts.tile([P, nt * n_experts], f32)
            idx_acc = consts.tile([P, nt * TOP_K], u32)
            gates_acc = consts.tile([P, nt * TOP_K], f32)

            for ph in range(n_pass):
                tw = tokens_per_pass
                ps = [psB.tile([n_experts, cg_w], f32, tag="psB",
                               name=f"psB_{ph}_{cg}") for cg in range(n_cg)]
                for kc in range(kc_n):
                    xt_t = xp.tile([P, tw], fmm, tag="xt")
                    nc.sync.dma_start(
                        out=xt_t[:],
                        in_=xt_d[kc * P:(kc + 1) * P, ph * tw:(ph + 1) * tw])
                    for cg in range(n_cg):
                        nc.tensor.matmul(
                            ps[cg][:],
                            lhsT=w_sb[:, kc * n_experts:(kc + 1) * n_experts],
                            rhs=xt_t[:, cg * cg_w:(cg + 1) * cg_w],
                            start=(kc == 0), stop=(kc == kc_n - 1))

                for cg in range(n_cg):
                    lgT = lgp.tile([n_experts, cg_w], f32, tag="lgT")
                    nc.vector.tensor_copy(lgT[:], ps[cg][:])
                    for tt in range(tpc):
                        g = ph * (tw // P) + cg * tpc + tt
                        pst = psT.tile([P, n_experts], f32, tag="psT",
                                       name=f"psT_{g}")
                        nc.tensor.transpose(
                            pst[:], lgT[:, tt * P:(tt + 1) * P], id_sb[:])
                        lg = logits_acc[:, g * n_experts:(g + 1) * n_experts]
                        nc.vector.tensor_add(lg, pst[:], b_sb[:])

                        maxv = small.tile([P, 8], f32, tag="maxv")
                        nc.vector.max(out=maxv[:], in_=lg)
                        idx8 = small.tile([P, 8], u32, tag="idx8")
                        nc.vector.max_index(out=idx8[:], in_max=maxv[:], in_values=lg)
                        nc.vector.tensor_copy(idx_acc[:, g * 2:g * 2 + 2], idx8[:, 0:2])

                        ed = small.tile([P, 1], f32, tag="ed")
                        nc.vector.tensor_sub(ed[:], maxv[:, 1:2], maxv[:, 0:1])
                        nc.scalar.activation(ed[:], ed[:], mybir.ActivationFunctionType.Exp)
                        s = small.tile([P, 1], f32, tag="s")
                        nc.vector.tensor_scalar_add(s[:], ed[:], 1.0)
                        g0 = gates_acc[:, g * 2:g * 2 + 1]
                        nc.vector.reciprocal(g0, s[:])
                        nc.vector.tensor_mul(gates_acc[:, g * 2 + 1:g * 2 + 2], ed[:], g0)

            if permuted_out:
                nc.sync.dma_start(out=logits_d[:], in_=logits_acc[:])
                nc.sync.dma_start(out=idx_d[:], in_=idx_acc[:])
                nc.sync.dma_start(out=gates_d[:], in_=gates_acc[:])
            else:
                nc.sync.dma_start(
                    out=logits_d.rearrange("(i p) e -> p i e", p=P),
                    in_=logits_acc[:].rearrange("p (i e) -> p i e", e=n_experts))
                nc.sync.dma_start(
                    out=idx_d.rearrange("(i p) e -> p i e", p=P),
                    in_=idx_acc[:].rearrange("p (i e) -> p i e", e=TOP_K))
                nc.sync.dma_start(
                    out=gates_d.rearrange("(i p) e -> p i e", p=P),
                    in_=gates_acc[:].rearrange("p (i e) -> p i e", e=TOP_K))

    nc.compile()
    return nc


def build_program(d_model=D_MODEL, t_core=2048, n_experts=NUM_EXPERTS,
                  tiles_per_pass=8, x_bufs=4):
    """Build the per-core Bass/Tile program. Returns the compiled Bacc module."""
    import concourse.bacc as bacc
    import concourse.tile as tile
    import concourse.mybir as mybir

    f32 = mybir.dt.float32
    u32 = mybir.dt.uint32

    kc_n = d_model // P          # contraction chunks
    nt = t_core // P             # token tiles per core
    tiles_per_pass = min(tiles_per_pass, nt)
    assert nt % tiles_per_pass == 0
    n_pass = nt // tiles_per_pass

    nc = bacc.Bacc("TRN2", target_bir_lowering=False, debug=False,
                   num_devices=N_CORES)

    xt_d = nc.dram_tensor("xt", [d_model, t_core], f32, kind="ExternalInput").ap()
    wp_d = nc.dram_tensor("wp", [P, kc_n * n_experts], f32, kind="ExternalInput").ap()
    b_d = nc.dram_tensor("b128", [P, n_experts], f32, kind="ExternalInput").ap()
    logits_d = nc.dram_tensor("logits", [t_core, n_experts], f32, kind="ExternalOutput").ap()
    idx_d = nc.dram_tensor("idx", [t_core, TOP_K], u32, kind="ExternalOutput").ap()
    gates_d = nc.dram_tensor("gates", [t_core, TOP_K], f32, kind="ExternalOutput").ap()

    with tile.TileContext(nc) as tc:
        with (
            tc.tile_pool(name="consts", bufs=1) as consts,
            tc.tile_pool(name="xp", bufs=x_bufs) as xp,
            tc.tile_pool(name="ps", bufs=tiles_per_pass, space="PSUM") as psp,
            tc.tile_pool(name="small", bufs=3) as small,
        ):
            w_sb = consts.tile([P, kc_n * n_experts], f32)
            nc.sync.dma_start(out=w_sb[:], in_=wp_d[:])
            b_sb = consts.tile([P, n_experts], f32)
            nc.sync.dma_start(out=b_sb[:], in_=b_d[:])

            logits_acc = consts.tile([P, nt * n_experts], f32)
            idx_acc = consts.tile([P, nt * TOP_K], u32)
            gates_acc = consts.tile([P, nt * TOP_K], f32)

            for ph in range(n_pass):
                tw = tiles_per_pass * P  # tokens in this pass
                ps_tiles = [psp.tile([P, n_experts], f32, tag="ps",
                                     name=f"ps_{ph}_{t}")
                            for t in range(tiles_per_pass)]
                for kc in range(kc_n):
                    xt_t = xp.tile([P, tw], f32, tag="xt")
                    nc.sync.dma_start(
                        out=xt_t[:],
                        in_=xt_d[kc * P:(kc + 1) * P, ph * tw:(ph + 1) * tw])
                    for t in range(tiles_per_pass):
                        nc.tensor.matmul(
                            ps_tiles[t][:],
                            lhsT=xt_t[:, t * P:(t + 1) * P],
                            rhs=w_sb[:, kc * n_experts:(kc + 1) * n_experts],
                            start=(kc == 0), stop=(kc == kc_n - 1))

                for t in range(tiles_per_pass):
                    g = ph * tiles_per_pass + t
                    lg = logits_acc[:, g * n_experts:(g + 1) * n_experts]
                    nc.vector.tensor_add(lg, ps_tiles[t][:], b_sb[:])

                    maxv = small.tile([P, 8], f32, tag="maxv")
                    nc.vector.max(out=maxv[:], in_=lg)
                    idx8 = small.tile([P, 8], u32, tag="idx8")
                    nc.vector.max_index(out=idx8[:], in_max=maxv[:], in_values=lg)
                    nc.vector.tensor_copy(idx_acc[:, g * 2:g * 2 + 2], idx8[:, 0:2])

                    # top-2 softmax: d = l1 - l0 (<= 0), e = exp(d)
                    # g0 = 1/(1+e), g1 = e/(1+e)
                    ed = small.tile([P, 1], f32, tag="ed")
                    nc.vector.tensor_sub(ed[:], maxv[:, 1:2], maxv[:, 0:1])
                    nc.scalar.activation(ed[:], ed[:], mybir.ActivationFunctionType.Exp)
                    s = small.tile([P, 1], f32, tag="s")
                    nc.vector.tensor_scalar_add(s[:], ed[:], 1.0)
                    g0 = gates_acc[:, g * 2:g * 2 + 1]
                    nc.vector.reciprocal(g0, s[:])
                    nc.vector.tensor_mul(gates_acc[:, g * 2 + 1:g * 2 + 2], ed[:], g0)

            # Batched stores: DRAM [(i p), e] <- SBUF [p, (i e)]
            nc.sync.dma_start(
                out=logits_d.rearrange("(i p) e -> p i e", p=P),
                in_=logits_acc[:].rearrange("p (i e) -> p i e", e=n_experts))
            nc.sync.dma_start(
                out=idx_d.rearrange("(i p) e -> p i e", p=P),
                in_=idx_acc[:].rearrange("p (i e) -> p i e", e=TOP_K))
            nc.sync.dma_start(
                out=gates_d.rearrange("(i p) e -> p i e", p=P),
                in_=gates_acc[:].rearrange("p (i e) -> p i e", e=TOP_K))

    nc.compile()
    return nc


def make_in_maps(x, w, b, d_model, t_core, n_experts, with_ident=False):
    """Host-side shard + layout prep. x: [tokens_total, d_model]."""
    kc_n = d_model // P
    wp = np.ascontiguousarray(
        w.reshape(kc_n, P, n_experts).transpose(1, 0, 2).reshape(P, kc_n * n_experts))
    b128 = np.ascontiguousarray(np.broadcast_to(b, (P, n_experts)), dtype=np.float32)
    ident = np.eye(n_experts, dtype=np.float32)
    in_maps = []
    for i in range(N_CORES):
        shard = x[i * t_core:(i + 1) * t_core]
        xt = np.ascontiguousarray(shard.T)
        m = {"xt": xt, "wp": wp, "b128": b128}
        if with_ident:
            m["ident"] = ident
        in_maps.append(m)
    return in_maps


_CACHE = {}


VARIANT = "b"


def _run(x, w, b, d_model, t_core, n_experts, trace=False, variant=None):
    from concourse.bass_utils import run_bass_kernel_spmd
    variant = VARIANT if variant is None else variant
    key = (variant, d_model, t_core, n_experts)
    if key not in _CACHE:
        if variant == "b":
            _CACHE[key] = build_program_b(d_model, t_core, n_experts)
        else:
            _CACHE[key] = build_program(d_model, t_core, n_experts)
    nc = _CACHE[key]
    in_maps = make_in_maps(x, w, b, d_model, t_core, n_experts,
                           with_ident=(variant == "b"))
    res = run_bass_kernel_spmd(nc, in_maps, core_ids=list(range(N_CORES)),
                               trace=trace)

    def unshuffle(a, width):
        # permuted store layout [128, nt*width]: token i*128+p sits at
        # row p, cols [i*width, (i+1)*width)
        if a.shape[0] == t_core:
            return a
        nt = a.shape[1] // width
        return a.reshape(P, nt, width).transpose(1, 0, 2).reshape(t_core, width)

    logits = np.concatenate(
        [unshuffle(res.results[i]["logits"], n_experts) for i in range(N_CORES)])
    idx = np.concatenate(
        [unshuffle(res.results[i]["idx"], TOP_K) for i in range(N_CORES)])
    gates = np.concatenate(
        [unshuffle(res.results[i]["gates"], TOP_K) for i in range(N_CORES)])
    return logits, idx.astype(np.int32), gates, res


def kernel(x, w, b):
    x = np.asarray(x, dtype=np.float32)
    w = np.asarray(w, dtype=np.float32)
    b = np.asarray(b, dtype=np.float32)
    bsz, seq, d_model = x.shape
    n_experts = w.shape[1]
    tokens = bsz * seq
    t_core = tokens // N_CORES
    logits, idx, gates, _ = _run(
        x.reshape(tokens, d_model), w, b, d_model, t_core, n_experts)
    return (logits.reshape(bsz, seq, n_experts),
            idx.reshape(bsz, seq, TOP_K),
            gates.reshape(bsz, seq, TOP_K))


# revision 15
# speedup vs baseline: 2.0981x; 1.1484x over previous
"""ExpertRouter (MoE routing) Trainium2 kernel.

Data-parallel over tokens across 8 NeuronCores. Each core computes, for its
2048-token shard: router logits x@w+b ([2048, 64] fp32), top-2 expert
indices, and softmax gates over the top-2 logits.

Host-side prep per shard: x is transposed to [d_model, tokens] so DMA loads
land [128 d_model, tokens] tiles directly (d_model on SBUF partitions = the
matmul contraction dim), avoiding any on-chip transpose. The tiny router
weight is replicated to all cores, packed [128, KC*64].

Per 128-token tile the PE accumulates 32 fp32 matmuls (K=128 each) into a
[128 tokens, 64 experts] PSUM tile; DVE adds bias, the DVE sort hardware
(max / max_index) yields descending top-8 values+indices, and gates come
from g0 = 1/(1+e^(l1-l0)), g1 = e^(l1-l0)*g0.
"""

import numpy as np

D_MODEL = 4096
NUM_EXPERTS = 64
TOP_K = 2
N_CORES = 8
P = 128  # partitions / matmul K-chunk / token-tile size


def build_program_b(d_model=D_MODEL, t_core=2048, n_experts=NUM_EXPERTS,
                    tokens_per_pass=1024, cg_w=512, x_bufs=8, use_f32r=False,
                    warmup_mms=20, permuted_out=True):
    """Design B: w stationary [128, 64] (reused), x moving at float32r full
    rate. PSUM gets logits^T [64, cg_w]; a small PE transpose flips each
    128-token block back to [128 tokens, 64 experts] for the top-k path."""
    import concourse.bacc as bacc
    import concourse.tile as tile
    import concourse.mybir as mybir

    f32 = mybir.dt.float32
    fmm = mybir.dt.float32r if use_f32r else f32
    u32 = mybir.dt.uint32

    kc_n = d_model // P
    nt = t_core // P
    assert t_core % tokens_per_pass == 0 and tokens_per_pass % cg_w == 0
    n_pass = t_core // tokens_per_pass
    n_cg = tokens_per_pass // cg_w
    tpc = cg_w // P  # token tiles per column group

    nc = bacc.Bacc("TRN2", target_bir_lowering=False, debug=False,
                   num_devices=N_CORES)

    bf16 = mybir.dt.bfloat16
    xt_d = nc.dram_tensor("xt", [d_model, t_core], fmm, kind="ExternalInput").ap()
    wp_d = nc.dram_tensor("wp", [P, kc_n * n_experts], fmm, kind="ExternalInput").ap()
    b_d = nc.dram_tensor("b128", [P, n_experts], f32, kind="ExternalInput").ap()
    id_d = nc.dram_tensor("ident", [n_experts, n_experts], f32, kind="ExternalInput").ap()
    if permuted_out:
        logits_d = nc.dram_tensor("logits", [P, nt * n_experts], f32, kind="ExternalOutput").ap()
        idx_d = nc.dram_tensor("idx", [P, nt * TOP_K], u32, kind="ExternalOutput").ap()
        gates_d = nc.dram_tensor("gates", [P, nt * TOP_K], f32, kind="ExternalOutput").ap()
    else:
        logits_d = nc.dram_tensor("logits", [t_core, n_experts], f32, kind="ExternalOutput").ap()
        idx_d = nc.dram_tensor("idx", [t_core, TOP_K], u32, kind="ExternalOutput").ap()
        gates_d = nc.dram_tensor("gates", [t_core, TOP_K], f32, kind="ExternalOutput").ap()

    with tile.TileContext(nc) as tc:
        with (
            tc.tile_pool(name="consts", bufs=1) as consts,
            tc.tile_pool(name="xp", bufs=x_bufs) as xp,
            tc.tile_pool(name="psB", bufs=2 * n_cg, space="PSUM") as psB,
            tc.tile_pool(name="psT", bufs=2, space="PSUM") as psT,
            tc.tile_pool(name="lgp", bufs=2) as lgp,
            tc.tile_pool(name="small", bufs=3) as small,
        ):
            if warmup_mms:
                # HAM pre-warm: ~20 bf16 MMs (~4us busy) ahead of the first
                # real matmul so fp32 streams run at 2.4 GHz from the start.
                wt = consts.tile([P, 512], bf16)
                nc.vector.memset(wt[:], 0.0)
                wps = psT.tile([P, 512], f32, tag="warm")
                for i in range(warmup_mms):
                    nc.tensor.matmul(wps[:], lhsT=wt[:, 0:P], rhs=wt[:],
                                     start=True, stop=True)
            w_sb = consts.tile([P, kc_n * n_experts], fmm)
            nc.sync.dma_start(out=w_sb[:], in_=wp_d[:])
            b_sb = consts.tile([P, n_experts], f32)
            nc.sync.dma_start(out=b_sb[:], in_=b_d[:])
            id_sb = consts.tile([n_experts, n_experts], f32)
            nc.sync.dma_start(out=id_sb[:], in_=id_d[:])

            logits_acc = consts.tile([P, nt * n_experts], f32)
            idx_acc = consts.tile([P, nt * TOP_K], u32)
            gates_acc = consts.tile([P, nt * TOP_K], f32)

            for ph in range(n_pass):
                tw = tokens_per_pass
                ps = [psB.tile([n_experts, cg_w], f32, tag="psB",
                               name=f"psB_{ph}_{cg}") for cg in range(n_cg)]
                for kc in range(kc_n):
                    xt_t = xp.tile([P, tw], fmm, tag="xt")
                    nc.sync.dma_start(
                        out=xt_t[:],
                        in_=xt_d[kc * P:(kc + 1) * P, ph * tw:(ph + 1) * tw])
                    for cg in range(n_cg):
                        nc.tensor.matmul(
                            ps[cg][:],
                            lhsT=w_sb[:, kc * n_experts:(kc + 1) * n_experts],
                            rhs=xt_t[:, cg * cg_w:(cg + 1) * cg_w],
                            start=(kc == 0), stop=(kc == kc_n - 1))

                for cg in range(n_cg):
                    lgT = lgp.tile([n_experts, cg_w], f32, tag="lgT")
                    nc.vector.tensor_copy(lgT[:], ps[cg][:])
                    for tt in range(tpc):
                        g = ph * (tw // P) + cg * tpc + tt
                        pst = psT.tile([P, n_experts], f32, tag="psT",
                                       name=f"psT_{g}")
                        nc.tensor.transpose(
                            pst[:], lgT[:, tt * P:(tt + 1) * P], id_sb[:])
                        lg = logits_acc[:, g * n_experts:(g + 1) * n_experts]
                        nc.vector.tensor_add(lg, pst[:], b_sb[:])

                        maxv = small.tile([P, 8], f32, tag="maxv")
                        nc.vector.max(out=maxv[:], in_=lg)
                        idx8 = small.tile([P, 8], u32, tag="idx8")
                        nc.vector.max_index(out=idx8[:], in_max=maxv[:], in_values=lg)
                        nc.vector.tensor_copy(idx_acc[:, g * 2:g * 2 + 2], idx8[:, 0:2])

                        ed = small.tile([P, 1], f32, tag="ed")
                        nc.vector.tensor_sub(ed[:], maxv[:, 1:2], maxv[:, 0:1])
                        nc.scalar.activation(ed[:], ed[:], mybir.ActivationFunctionType.Exp)
                        s = small.tile([P, 1], f32, tag="s")
                        nc.vector.tensor_scalar_add(s[:], ed[:], 1.0)
                        g0 = gates_acc[:, g * 2:g * 2 + 1]
                        nc.vector.reciprocal(g0, s[:])
                        nc.vector.tensor_mul(gates_acc[:, g * 2 + 1:g * 2 + 2], ed[:], g0)

            if permuted_out:
                nc.sync.dma_start(out=logits_d[:], in_=logits_acc[:])
                nc.sync.dma_start(out=idx_d[:], in_=idx_acc[:])
                nc.sync.dma_start(out=gates_d[:], in_=gates_acc[:])
            else:
                nc.sync.dma_start(
                    out=logits_d.rearrange("(i p) e -> p i e", p=P),
                    in_=logits_acc[:].rearrange("p (i e) -> p i e", e=n_experts))
                nc.sync.dma_start(
                    out=idx_d.rearrange("(i p) e -> p i e", p=P),
                    in_=idx_acc[:].rearrange("p (i e) -> p i e", e=TOP_K))
                nc.sync.dma_start(
                    out=gates_d.rearrange("(i p) e -> p i e", p=P),
                    in_=gates_acc[:].rearrange("p (i e) -> p i e", e=TOP_K))

    nc.compile()
    return nc


X_SCALE = 16.0
W_SCALE = 64.0
OUT_SCALE = 1.0 / (X_SCALE * W_SCALE)


def build_program_c(d_model=D_MODEL, t_core=2048, n_experts=NUM_EXPERTS,
                    tokens_per_pass=1024, cg_w=512, x_bufs=6, warmup_mms=20):
    """fp16x3: host splits 16*x = xh+xl and 64*w = wh+wl into fp16 pairs.
    logits*1024 = wh@xh + wh@xl + wl@xh (xl@wl term ~2^-22, dropped), all
    fp16 matmuls at 1 cyc/row (vs fp32's 4). Same 32 MB HBM traffic.
    Epilogue rescales by 2^-10 during the bias add."""
    import concourse.bacc as bacc
    import concourse.tile as tile
    import concourse.mybir as mybir

    f32 = mybir.dt.float32
    f16 = mybir.dt.float16
    bf16 = mybir.dt.bfloat16
    u32 = mybir.dt.uint32

    kc_n = d_model // P
    nt = t_core // P
    assert t_core % tokens_per_pass == 0 and tokens_per_pass % cg_w == 0
    n_pass = t_core // tokens_per_pass
    n_cg = tokens_per_pass // cg_w
    tpc = cg_w // P

    nc = bacc.Bacc("TRN2", target_bir_lowering=False, debug=False,
                   num_devices=N_CORES)

    xth_d = nc.dram_tensor("xth", [d_model, t_core], f16, kind="ExternalInput").ap()
    xtl_d = nc.dram_tensor("xtl", [d_model, t_core], f16, kind="ExternalInput").ap()
    wh_d = nc.dram_tensor("wh", [P, kc_n * n_experts], f16, kind="ExternalInput").ap()
    wl_d = nc.dram_tensor("wl", [P, kc_n * n_experts], f16, kind="ExternalInput").ap()
    b_d = nc.dram_tensor("b128", [P, n_experts], f32, kind="ExternalInput").ap()
    id_d = nc.dram_tensor("ident", [n_experts, n_experts], f32, kind="ExternalInput").ap()
    logits_d = nc.dram_tensor("logits", [P, nt * n_experts], f32, kind="ExternalOutput").ap()
    idx_d = nc.dram_tensor("idx", [P, nt * TOP_K], u32, kind="ExternalOutput").ap()
    gates_d = nc.dram_tensor("gates", [P, nt * TOP_K], f32, kind="ExternalOutput").ap()

    with tile.TileContext(nc) as tc:
        with (
            tc.tile_pool(name="consts", bufs=1) as consts,
            tc.tile_pool(name="xp", bufs=x_bufs) as xp,
            tc.tile_pool(name="psB", bufs=2 * n_cg, space="PSUM") as psB,
            tc.tile_pool(name="psT", bufs=2, space="PSUM") as psT,
            tc.tile_pool(name="lgp", bufs=2) as lgp,
            tc.tile_pool(name="small", bufs=3) as small,
        ):
            if warmup_mms:
                wt = consts.tile([P, 512], bf16)
                nc.vector.memset(wt[:], 0.0)
                wps = psT.tile([P, 512], f32, tag="warm")
                for i in range(warmup_mms):
                    nc.tensor.matmul(wps[:], lhsT=wt[:, 0:P], rhs=wt[:],
                                     start=True, stop=True)
            wh_sb = consts.tile([P, kc_n * n_experts], f16)
            nc.sync.dma_start(out=wh_sb[:], in_=wh_d[:])
            wl_sb = consts.tile([P, kc_n * n_experts], f16)
            nc.sync.dma_start(out=wl_sb[:], in_=wl_d[:])
            b_sb = consts.tile([P, n_experts], f32)
            nc.sync.dma_start(out=b_sb[:], in_=b_d[:])
            id_sb = consts.tile([n_experts, n_experts], f32)
            nc.sync.dma_start(out=id_sb[:], in_=id_d[:])

            logits_acc = consts.tile([P, nt * n_experts], f32)
            idx_acc = consts.tile([P, nt * TOP_K], u32)
            gates_acc = consts.tile([P, nt * TOP_K], f32)

            for ph in range(n_pass):
                tw = tokens_per_pass
                ps = [psB.tile([n_experts, cg_w], f32, tag="psB",
                               name=f"psB_{ph}_{cg}") for cg in range(n_cg)]
                for kc in range(kc_n):
                    xh_t = xp.tile([P, tw], f16, tag="xh")
                    nc.sync.dma_start(
                        out=xh_t[:],
                        in_=xth_d[kc * P:(kc + 1) * P, ph * tw:(ph + 1) * tw])
                    xl_t = xp.tile([P, tw], f16, tag="xl")
                    nc.sync.dma_start(
                        out=xl_t[:],
                        in_=xtl_d[kc * P:(kc + 1) * P, ph * tw:(ph + 1) * tw])
                    wslice = slice(kc * n_experts, (kc + 1) * n_experts)
                    # wh stationary reused across 2*n_cg streams, then wl
                    for cg in range(n_cg):
                        nc.tensor.matmul(
                            ps[cg][:], lhsT=wh_sb[:, wslice],
                            rhs=xh_t[:, cg * cg_w:(cg + 1) * cg_w],
                            start=(kc == 0), stop=False)
                    for cg in range(n_cg):
                        nc.tensor.matmul(
                            ps[cg][:], lhsT=wh_sb[:, wslice],
                            rhs=xl_t[:, cg * cg_w:(cg + 1) * cg_w],
                            start=False, stop=False)
                    for cg in range(n_cg):
                        nc.tensor.matmul(
                            ps[cg][:], lhsT=wl_sb[:, wslice],
                            rhs=xh_t[:, cg * cg_w:(cg + 1) * cg_w],
                            start=False, stop=(kc == kc_n - 1))

                for cg in range(n_cg):
                    lgT = lgp.tile([n_experts, cg_w], f32, tag="lgT")
                    nc.vector.tensor_copy(lgT[:], ps[cg][:])
                    for tt in range(tpc):
                        g = ph * (tw // P) + cg * tpc + tt
                        pst = psT.tile([P, n_experts], f32, tag="psT",
                                       name=f"psT_{g}")
                        nc.tensor.transpose(
                            pst[:], lgT[:, tt * P:(tt + 1) * P], id_sb[:])
                        lg = logits_acc[:, g * n_experts:(g + 1) * n_experts]
                        nc.vector.scalar_tensor_tensor(
                            lg, pst[:], OUT_SCALE, b_sb[:],
                            op0=mybir.AluOpType.mult, op1=mybir.AluOpType.add)

                        maxv = small.tile([P, 8], f32, tag="maxv")
                        nc.vector.max(out=maxv[:], in_=lg)
                        idx8 = small.tile([P, 8], u32, tag="idx8")
                        nc.vector.max_index(out=idx8[:], in_max=maxv[:], in_values=lg)
                        nc.vector.tensor_copy(idx_acc[:, g * 2:g * 2 + 2], idx8[:, 0:2])

                        ed = small.tile([P, 1], f32, tag="ed")
                        nc.vector.tensor_sub(ed[:], maxv[:, 1:2], maxv[:, 0:1])
                        nc.scalar.activation(ed[:], ed[:], mybir.ActivationFunctionType.Exp)
                        s = small.tile([P, 1], f32, tag="s")
                        nc.vector.tensor_scalar_add(s[:], ed[:], 1.0)
                        g0 = gates_acc[:, g * 2:g * 2 + 1]
                        nc.vector.reciprocal(g0, s[:])
                        nc.vector.tensor_mul(gates_acc[:, g * 2 + 1:g * 2 + 2], ed[:], g0)

            nc.sync.dma_start(out=logits_d[:], in_=logits_acc[:])
            nc.sync.dma_start(out=idx_d[:], in_=idx_acc[:])
            nc.sync.dma_start(out=gates_d[:], in_=gates_acc[:])

    nc.compile()
    return nc


def make_in_maps_c(x, w, b, d_model, t_core, n_experts):
    kc_n = d_model // P
    ws = (w.astype(np.float64) * W_SCALE).astype(np.float32)
    wh = ws.astype(np.float16)
    wl = (ws - wh.astype(np.float32)).astype(np.float16)

    def pack(a):
        return np.ascontiguousarray(
            a.reshape(kc_n, P, n_experts).transpose(1, 0, 2).reshape(P, kc_n * n_experts))

    wh_p, wl_p = pack(wh), pack(wl)
    b128 = np.ascontiguousarray(np.broadcast_to(b, (P, n_experts)), dtype=np.float32)
    ident = np.eye(n_experts, dtype=np.float32)

    xs = x * np.float32(X_SCALE)
    xh = xs.astype(np.float16)
    xl = (xs - xh.astype(np.float32)).astype(np.float16)
    in_maps = []
    for i in range(N_CORES):
        sl = slice(i * t_core, (i + 1) * t_core)
        in_maps.append({
            "xth": np.ascontiguousarray(xh[sl].T),
            "xtl": np.ascontiguousarray(xl[sl].T),
            "wh": wh_p, "wl": wl_p, "b128": b128, "ident": ident,
        })
    return in_maps


def build_program(d_model=D_MODEL, t_core=2048, n_experts=NUM_EXPERTS,
                  tiles_per_pass=8, x_bufs=4):
    """Build the per-core Bass/Tile program. Returns the compiled Bacc module."""
    import concourse.bacc as bacc
    import concourse.tile as tile
    import concourse.mybir as mybir

    f32 = mybir.dt.float32
    u32 = mybir.dt.uint32

    kc_n = d_model // P          # contraction chunks
    nt = t_core // P             # token tiles per core
    tiles_per_pass = min(tiles_per_pass, nt)
    assert nt % tiles_per_pass == 0
    n_pass = nt // tiles_per_pass

    nc = bacc.Bacc("TRN2", target_bir_lowering=False, debug=False,
                   num_devices=N_CORES)

    xt_d = nc.dram_tensor("xt", [d_model, t_core], f32, kind="ExternalInput").ap()
    wp_d = nc.dram_tensor("wp", [P, kc_n * n_experts], f32, kind="ExternalInput").ap()
    b_d = nc.dram_tensor("b128", [P, n_experts], f32, kind="ExternalInput").ap()
    logits_d = nc.dram_tensor("logits", [t_core, n_experts], f32, kind="ExternalOutput").ap()
    idx_d = nc.dram_tensor("idx", [t_core, TOP_K], u32, kind="ExternalOutput").ap()
    gates_d = nc.dram_tensor("gates", [t_core, TOP_K], f32, kind="ExternalOutput").ap()

    with tile.TileContext(nc) as tc:
        with (
            tc.tile_pool(name="consts", bufs=1) as consts,
            tc.tile_pool(name="xp", bufs=x_bufs) as xp,
            tc.tile_pool(name="ps", bufs=tiles_per_pass, space="PSUM") as psp,
            tc.tile_pool(name="small", bufs=3) as small,
        ):
            w_sb = consts.tile([P, kc_n * n_experts], f32)
            nc.sync.dma_start(out=w_sb[:], in_=wp_d[:])
            b_sb = consts.tile([P, n_experts], f32)
            nc.sync.dma_start(out=b_sb[:], in_=b_d[:])

            logits_acc = consts.tile([P, nt * n_experts], f32)
            idx_acc = consts.tile([P, nt * TOP_K], u32)
            gates_acc = consts.tile([P, nt * TOP_K], f32)

            for ph in range(n_pass):
                tw = tiles_per_pass * P  # tokens in this pass
                ps_tiles = [psp.tile([P, n_experts], f32, tag="ps",
                                     name=f"ps_{ph}_{t}")
                            for t in range(tiles_per_pass)]
                for kc in range(kc_n):
                    xt_t = xp.tile([P, tw], f32, tag="xt")
                    nc.sync.dma_start(
                        out=xt_t[:],
                        in_=xt_d[kc * P:(kc + 1) * P, ph * tw:(ph + 1) * tw])
                    for t in range(tiles_per_pass):
                        nc.tensor.matmul(
                            ps_tiles[t][:],
                            lhsT=xt_t[:, t * P:(t + 1) * P],
                            rhs=w_sb[:, kc * n_experts:(kc + 1) * n_experts],
                            start=(kc == 0), stop=(kc == kc_n - 1))

                for t in range(tiles_per_pass):
                    g = ph * tiles_per_pass + t
                    lg = logits_acc[:, g * n_experts:(g + 1) * n_experts]
                    nc.vector.tensor_add(lg, ps_tiles[t][:], b_sb[:])

                    maxv = small.tile([P, 8], f32, tag="maxv")
                    nc.vector.max(out=maxv[:], in_=lg)
                    idx8 = small.tile([P, 8], u32, tag="idx8")
                    nc.vector.max_index(out=idx8[:], in_max=maxv[:], in_values=lg)
                    nc.vector.tensor_copy(idx_acc[:, g * 2:g * 2 + 2], idx8[:, 0:2])

                    # top-2 softmax: d = l1 - l0 (<= 0), e = exp(d)
                    # g0 = 1/(1+e), g1 = e/(1+e)
                    ed = small.tile([P, 1], f32, tag="ed")
                    nc.vector.tensor_sub(ed[:], maxv[:, 1:2], maxv[:, 0:1])
                    nc.scalar.activation(ed[:], ed[:], mybir.ActivationFunctionType.Exp)
                    s = small.tile([P, 1], f32, tag="s")
                    nc.vector.tensor_scalar_add(s[:], ed[:], 1.0)
                    g0 = gates_acc[:, g * 2:g * 2 + 1]
                    nc.vector.reciprocal(g0, s[:])
                    nc.vector.tensor_mul(gates_acc[:, g * 2 + 1:g * 2 + 2], ed[:], g0)

            # Batched stores: DRAM [(i p), e] <- SBUF [p, (i e)]
            nc.sync.dma_start(
                out=logits_d.rearrange("(i p) e -> p i e", p=P),
                in_=logits_acc[:].rearrange("p (i e) -> p i e", e=n_experts))
            nc.sync.dma_start(
                out=idx_d.rearrange("(i p) e -> p i e", p=P),
                in_=idx_acc[:].rearrange("p (i e) -> p i e", e=TOP_K))
            nc.sync.dma_start(
                out=gates_d.rearrange("(i p) e -> p i e", p=P),
                in_=gates_acc[:].rearrange("p (i e) -> p i e", e=TOP_K))

    nc.compile()
    return nc


def make_in_maps(x, w, b, d_model, t_core, n_experts, with_ident=False):
    """Host-side shard + layout prep. x: [tokens_total, d_model]."""
    kc_n = d_model // P
    wp = np.ascontiguousarray(
        w.reshape(kc_n, P, n_experts).transpose(1, 0, 2).reshape(P, kc_n * n_experts))
    b128 = np.ascontiguousarray(np.broadcast_to(b, (P, n_experts)), dtype=np.float32)
    ident = np.eye(n_experts, dtype=np.float32)
    in_maps = []
    for i in range(N_CORES):
        shard = x[i * t_core:(i + 1) * t_core]
        xt = np.ascontiguousarray(shard.T)
        m = {"xt": xt, "wp": wp, "b128": b128}
        if with_ident:
            m["ident"] = ident
        in_maps.append(m)
    return in_maps


_CACHE = {}


VARIANT = "c"


def _run(x, w, b, d_model, t_core, n_experts, trace=False, variant=None):
    from concourse.bass_utils import run_bass_kernel_spmd
    variant = VARIANT if variant is None else variant
    key = (variant, d_model, t_core, n_experts)
    if key not in _CACHE:
        if variant == "c":
            _CACHE[key] = build_program_c(d_model, t_core, n_experts)
        elif variant == "b":
            _CACHE[key] = build_program_b(d_model, t_core, n_experts)
        else:
            _CACHE[key] = build_program(d_model, t_core, n_experts)
    nc = _CACHE[key]
    if variant == "c":
        in_maps = make_in_maps_c(x, w, b, d_model, t_core, n_experts)
    else:
        in_maps = make_in_maps(x, w, b, d_model, t_core, n_experts,
                               with_ident=(variant == "b"))
    res = run_bass_kernel_spmd(nc, in_maps, core_ids=list(range(N_CORES)),
                               trace=trace)

    def unshuffle(a, width):
        # permuted store layout [128, nt*width]: token i*128+p sits at
        # row p, cols [i*width, (i+1)*width)
        if a.shape[0] == t_core:
            return a
        nt = a.shape[1] // width
        return a.reshape(P, nt, width).transpose(1, 0, 2).reshape(t_core, width)

    logits = np.concatenate(
        [unshuffle(res.results[i]["logits"], n_experts) for i in range(N_CORES)])
    idx = np.concatenate(
        [unshuffle(res.results[i]["idx"], TOP_K) for i in range(N_CORES)])
    gates = np.concatenate(
        [unshuffle(res.results[i]["gates"], TOP_K) for i in range(N_CORES)])
    return logits, idx.astype(np.int32), gates, res


def kernel(x, w, b):
    x = np.asarray(x, dtype=np.float32)
    w = np.asarray(w, dtype=np.float32)
    b = np.asarray(b, dtype=np.float32)
    bsz, seq, d_model = x.shape
    n_experts = w.shape[1]
    tokens = bsz * seq
    t_core = tokens // N_CORES
    logits, idx, gates, _ = _run(
        x.reshape(tokens, d_model), w, b, d_model, t_core, n_experts)
    return (logits.reshape(bsz, seq, n_experts),
            idx.reshape(bsz, seq, TOP_K),
            gates.reshape(bsz, seq, TOP_K))
